# revision 1
# baseline (speedup 1.0000x reference)
"""CosineGraphAttentionLayer Trainium2 kernel (8-core SPMD, full I/O).

out = softmax(beta * cos_sim(xi, xj) + adj_mask) @ xj,  shapes:
  xi [8192,128] f32, xj [8192,128] f32, adj [8192,8192] int32, beta [1] f32.

Sharding: row-shard xi/adj across 8 cores (1024 rows each), xj replicated;
softmax rows are fully local; host concatenates per-core outputs.

Per-core pipeline (all matmuls fp16 operands, f32 PSUM accumulation):
  - normalize xi*beta/||xi||, xj/||xj|| on device (f32, Newton-refined rsqrt),
    cast fp16, PE-transpose -> uT [d,1024], vT [d,8192]
  - per m-chunk of 512: S = uT.T@vT chunk (PSUM f32) -> ACT Exp -> fp16 E;
    mask E *= adj16 (adj DMA'd int32, gpsimd-cast fp16); rowsum partials via
    tensor_scalar accum; PE-transpose E -> E^T; mm2: out2T += xj16.T @ E^T
  - finale: out = (out2T / rowsum).T, DMA out.
"""
import numpy as np

import concourse.mybir as mybir
import concourse.tile as tile
from concourse import bacc
from concourse.masks import make_identity
from concourse.bass_utils import run_bass_kernel_spmd

dt = mybir.dt
F16 = dt.float16
F32 = dt.float32
AX = mybir.AxisListType.X
MULT = mybir.AluOpType.mult
ADD = mybir.AluOpType.add
Act = mybir.ActivationFunctionType

N_CORES = 8
N, M, D = 8192, 8192, 128
NB = N // N_CORES          # 1024 rows per core
NT = NB // 128             # 8 n-tiles
MTILES = M // 128          # 64
MC = M // 512              # 16 m-chunks of 512
EPS = 1e-07


def build(reps=1, nb=NB, m=M):
    nt = nb // 128
    mc_n = m // 512
    mtiles = m // 128
    nc = bacc.Bacc("TRN2", target_bir_lowering=False, debug=False,
                   num_devices=N_CORES)
    xi = nc.dram_tensor("xi", [nb, D], F32, kind="ExternalInput")
    xj = nc.dram_tensor("xj", [m, D], F32, kind="ExternalInput")
    adj = nc.dram_tensor("adj", [nb, m], dt.int32, kind="ExternalInput")
    beta = nc.dram_tensor("beta", [1], F32, kind="ExternalInput")
    out = nc.dram_tensor("out", [nb, D], F32, kind="ExternalOutput")

    xi_v = xi.ap().rearrange("(t p) d -> p t d", p=128)    # [128, nt, 128]
    xj_v = xj.ap().rearrange("(t p) d -> p t d", p=128)    # [128, mtiles, 128]
    adj_v = adj.ap().rearrange("(t p) m -> p t m", p=128)  # [128, nt, m]
    out_v = out.ap().rearrange("(t p) d -> p t d", p=128)

    with tile.TileContext(nc) as tc:
        with (
            tc.tile_pool(name="const", bufs=1) as cpool,
            tc.tile_pool(name="persist", bufs=1) as pp,
            tc.tile_pool(name="psS", bufs=2, space="PSUM") as psS,
            tc.tile_pool(name="psET", bufs=4, space="PSUM") as psET,
            tc.tile_pool(name="psO", bufs=1, space="PSUM") as psO,
        ):
            ident16 = cpool.tile([128, 128], F16)
            make_identity(nc, ident16[:])
            ident32 = cpool.tile([128, 128], F32)
            make_identity(nc, ident32[:])

            uT = pp.tile([128, nb], F16)           # normalized beta*xi, transposed
            vT = pp.tile([128, m], F16)            # normalized xj, transposed
            xj16 = pp.tile([128, mtiles, 128], F16)  # raw xj fp16, natural tiles

            # ---------------- setup: norms, scaling, transposes ----------------
            with tc.tile_pool(name="setup", bufs=1) as sp:
                xi_sb = sp.tile([128, nt, 128], F32)
                nc.sync.dma_start(xi_sb[:], xi_v)
                xj_sb = sp.tile([128, mtiles, 128], F32)
                nc.sync.dma_start(xj_sb[:], xj_v)
                beta_sb = sp.tile([1, 1], F32)
                nc.sync.dma_start(beta_sb[0:1, 0:1], beta.ap()[0:1])
                beta_bc = sp.tile([128, 1], F32)
                nc.gpsimd.partition_broadcast(beta_bc[:], beta_sb[0:1, :])

                nc.vector.tensor_copy(xj16[:], xj_sb[:])  # f32 -> fp16

                ntot = nt + mtiles
                q = sp.tile([128, ntot], F32)   # sum of squares per row
                sq = sp.tile([128, mtiles, 128], F32)
                nc.scalar.activation(sq[:, 0:nt, :], xi_sb[:], Act.Square)
                for t in range(nt):
                    nc.vector.reduce_sum(q[:, t:t + 1], sq[:, t, :], axis=AX)
                nc.scalar.activation(sq[:], xj_sb[:], Act.Square)
                for t in range(mtiles):
                    nc.vector.reduce_sum(q[:, nt + t:nt + t + 1], sq[:, t, :], axis=AX)

                # y ~= 1/sqrt(q): reciprocal (exact-ish) + sqrt + 2x Newton
                r = sp.tile([128, ntot], F32)
                nc.vector.reciprocal(r[:], q[:])
                y = sp.tile([128, ntot], F32)
                nc.scalar.activation(y[:], r[:], Act.Sqrt)
                t1 = sp.tile([128, ntot], F32)
                t3 = sp.tile([128, ntot], F32)
                for _ in range(2):
                    nc.vector.tensor_mul(t1[:], y[:], y[:])
                    nc.vector.tensor_mul(t1[:], t1[:], q[:])
                    nc.vector.tensor_scalar(out=t3[:], in0=t1[:], scalar1=-0.5,
                                            scalar2=1.5, op0=MULT, op1=ADD)
                    nc.vector.tensor_mul(y[:], y[:], t3[:])
                # fold beta into xi scales
                nc.vector.tensor_scalar(out=y[:, 0:nt], in0=y[:, 0:nt],
                                        scalar1=beta_bc[:, 0:1], scalar2=None,
                                        op0=MULT)

                # scale rows to fp16, then PE-transpose into uT / vT
                u16 = sp.tile([128, nt, 128], F16)
                v16 = sp.tile([128, mtiles, 128], F16)
                for t in range(nt):
                    nc.vector.tensor_scalar(out=u16[:, t, :], in0=xi_sb[:, t, :],
                                            scalar1=y[:, t:t + 1], scalar2=None,
                                            op0=MULT)
                for t in range(mtiles):
                    nc.vector.tensor_scalar(out=v16[:, t, :], in0=xj_sb[:, t, :],
                                            scalar1=y[:, nt + t:nt + t + 1],
                                            scalar2=None, op0=MULT)
                for dst, src, ktiles in ((uT, u16, nt), (vT, v16, mtiles)):
                    for base in range(0, ktiles, 4):
                        jn = min(4, ktiles - base)
                        tp = psS.tile([128, 512], F16, tag="s")
                        for j in range(jn):
                            nc.tensor.transpose(tp[:, j * 128:(j + 1) * 128],
                                                src[:, base + j, :], ident16[:])
                        nc.vector.tensor_copy(
                            dst[:, base * 128:(base + jn) * 128],
                            tp[:, 0:jn * 128])

            # ---------------- main loop ----------------
            with (
                tc.tile_pool(name="adji", bufs=2) as adjip,
                tc.tile_pool(name="adjf", bufs=2) as adjfp,
                tc.tile_pool(name="em", bufs=2) as emp,
                tc.tile_pool(name="ets", bufs=8) as etsp,
                tc.tile_pool(name="fin", bufs=2) as finp,
                tc.tile_pool(name="rsp", bufs=1) as rspool,
            ):
                for rep in range(reps):
                    rs_parts = rspool.tile([128, nt * mc_n], F32, tag="rsp")
                    out2T = psO.tile([128, nb], F32, tag="o2")  # [d, n] accum
                    for mc in range(mc_n):
                        adj_i32 = adjip.tile([128, nt, 512], dt.int32, tag="ai")
                        nc.sync.dma_start(adj_i32[:],
                                          adj_v[:, :, mc * 512:(mc + 1) * 512])
                        adj16 = adjfp.tile([128, nt, 512], F16, tag="af")
                        nc.gpsimd.tensor_copy(adj16[:], adj_i32[:])

                        em_all = emp.tile([128, nt, 512], F16, tag="em")
                        for t in range(nt):
                            s_ps = psS.tile([128, 512], F32, tag="s")
                            nc.tensor.matmul(s_ps[:], uT[:, t * 128:(t + 1) * 128],
                                             vT[:, mc * 512:(mc + 1) * 512],
                                             start=True, stop=True)
                            nc.scalar.activation(em_all[:, t, :], s_ps[:], Act.Exp)
                            nc.vector.tensor_mul(em_all[:, t, :], em_all[:, t, :],
                                                 adj16[:, t, :])
                            nc.vector.tensor_scalar(
                                out=em_all[:, t, :], in0=em_all[:, t, :],
                                scalar1=1.0, scalar2=0.0, op0=MULT, op1=ADD,
                                accum_out=rs_parts[:, t * mc_n + mc:t * mc_n + mc + 1])
                        for j in range(4):
                            et_ps = psET.tile([128, nb], F16, tag="et")
                            for t in range(nt):
                                nc.tensor.transpose(et_ps[:, t * 128:(t + 1) * 128],
                                                    em_all[:, t, j * 128:(j + 1) * 128],
                                                    ident16[:])
                            et_sb = etsp.tile([128, nb], F16, tag="ets")
                            nc.vector.tensor_copy(et_sb[:], et_ps[:])
                            mt = mc * 4 + j
                            hw_ = min(512, nb)
                            for h in range(nb // hw_):
                                nc.tensor.matmul(out2T[:, h * hw_:(h + 1) * hw_],
                                                 xj16[:, mt, :],
                                                 et_sb[:, h * hw_:(h + 1) * hw_],
                                                 start=(mt == 0),
                                                 stop=(mt == mtiles - 1))

                    # ---------------- finale ----------------
                    rs = finp.tile([128, nt], F32, tag="rs")
                    for t in range(nt):
                        nc.vector.reduce_sum(rs[:, t:t + 1],
                                             rs_parts[:, t * mc_n:(t + 1) * mc_n],
                                             axis=AX)
                    rrs = finp.tile([128, nt], F32, tag="rrs")
                    nc.vector.reciprocal(rrs[:], rs[:])
                    o2_sb = finp.tile([128, nb], F32, tag="o2sb")
                    nc.vector.tensor_copy(o2_sb[:], out2T[:])
                    out_sb = finp.tile([128, nt, 128], F32, tag="osb")
                    for t in range(nt):
                        ot_ps = psS.tile([128, 128], F32, tag="s")
                        nc.tensor.transpose(ot_ps[:], o2_sb[:, t * 128:(t + 1) * 128],
                                            ident32[:])
                        nc.vector.tensor_scalar(out=out_sb[:, t, :], in0=ot_ps[:],
                                                scalar1=rrs[:, t:t + 1],
                                                scalar2=None, op0=MULT)
                    nc.sync.dma_start(out_v, out_sb[:])
    nc.compile()
    return nc


_NC_CACHE = {}


def _get_nc(reps=1):
    if reps not in _NC_CACHE:
        _NC_CACHE[reps] = build(reps=reps)
    return _NC_CACHE[reps]


def kernel(xi, xj, adj, beta):
    xi = np.ascontiguousarray(np.asarray(xi, dtype=np.float32))
    xj = np.ascontiguousarray(np.asarray(xj, dtype=np.float32))
    adj = np.ascontiguousarray(np.asarray(adj, dtype=np.int32))
    beta = np.ascontiguousarray(np.asarray(beta, dtype=np.float32))
    nc = _get_nc(reps=1)
    in_maps = []
    for c in range(N_CORES):
        sl = slice(c * NB, (c + 1) * NB)
        in_maps.append({
            "xi": np.ascontiguousarray(xi[sl]),
            "xj": xj,
            "adj": np.ascontiguousarray(adj[sl]),
            "beta": beta,
        })
    res = run_bass_kernel_spmd(nc, in_maps, core_ids=list(range(N_CORES)))
    return np.concatenate([res.results[c]["out"] for c in range(N_CORES)], axis=0)

